# revision 5
# baseline (speedup 1.0000x reference)
"""BitNet MLP (ternary-quantized SwiGLU) on 8 Trainium2 NeuronCores.

Strategy: tensor-parallel over hidden_dim. Each core owns a 1/8 slice of
gate/up rows and the matching down_proj columns. Activations are kept in
transposed layout [feature, token] on device so every matmul contracts over
the partition dimension with no on-device transposes. Weights are ternarized
on device (mask via fused abs/is_gt on DVE, sign on ACT) into bf16; matmuls
run in bf16 with fp32 PSUM accumulation. The down-proj partial sums are
ReduceScattered across the 8 cores in token chunks, overlapping the
collective with the remaining compute.
"""

import sys

sys.path.insert(0, "/opt/trn_rl_repo")

import numpy as np
import ml_dtypes

BF16 = ml_dtypes.bfloat16
NCORES = 8
P = 128

_CACHE = {}


def _build(d, t_total, h_total, dim):
    """Build + finalize the SPMD Bass module for the given full dims."""
    import concourse.mybir as mybir
    import concourse.tile as tile
    from concourse import bacc

    f32 = mybir.dt.float32
    bf16 = mybir.dt.bfloat16

    h_local = h_total // NCORES
    dim_shard = dim // NCORES

    T_CHUNK = 512 if t_total % 512 == 0 else 256
    H_SLAB = 256 if h_local % 256 == 0 else 128
    D_SLAB = 512 if dim % 512 == 0 else dim

    n_tc = t_total // T_CHUNK
    n_slab = h_local // H_SLAB
    ht_per_slab = H_SLAB // P
    n_ko = d // P            # contraction tiles for gate/up (over d)
    n_ho = h_local // P      # contraction tiles for down (over h_local)
    n_dslab = dim // D_SLAB
    dt_per_dslab = D_SLAB // P
    n_oo = dim_shard // P    # output row tiles per core

    assert t_total % T_CHUNK == 0 and h_local % H_SLAB == 0
    assert d % P == 0 and dim % D_SLAB == 0 and D_SLAB % P == 0
    assert dim_shard % P == 0

    nc = bacc.Bacc("TRN2", target_bir_lowering=False, debug=False)

    xT_e = nc.dram_tensor("xT", [d, t_total], bf16, kind="ExternalInput")
    gwT_e = nc.dram_tensor("gwT", [d, h_local], f32, kind="ExternalInput")
    uwT_e = nc.dram_tensor("uwT", [d, h_local], f32, kind="ExternalInput")
    dwT_e = nc.dram_tensor("dwT", [h_local, dim], f32, kind="ExternalInput")
    gs_e = nc.dram_tensor("gs", [h_local, 1], f32, kind="ExternalInput")
    us_e = nc.dram_tensor("us", [h_local, 1], f32, kind="ExternalInput")
    ds_e = nc.dram_tensor("ds", [dim_shard, 1], f32, kind="ExternalInput")
    thr_e = nc.dram_tensor("thr", [P, 3], f32, kind="ExternalInput")
    out_e = nc.dram_tensor("out", [dim_shard, t_total], f32, kind="ExternalOutput")

    with tile.TileContext(nc) as tc:
        with (
            tc.tile_pool(name="const", bufs=1) as constp,
            tc.tile_pool(name="dram", bufs=1, space="DRAM") as dram,
        ):
            thr_sb = constp.tile([P, 3], f32)
            nc.sync.dma_start(thr_sb[:], thr_e[:])
            gs_sb = constp.tile([P, n_ho], f32)
            nc.sync.dma_start(gs_sb[:], gs_e[:].rearrange("(o p) u -> p (o u)", p=P))
            us_sb = constp.tile([P, n_ho], f32)
            nc.sync.dma_start(us_sb[:], us_e[:].rearrange("(o p) u -> p (o u)", p=P))
            ds_sb = constp.tile([P, n_oo], f32)
            nc.sync.dma_start(ds_sb[:], ds_e[:].rearrange("(o p) u -> p (o u)", p=P))

            hid = dram.tile([h_local, t_total], bf16)
            dwq = dram.tile([h_local, dim], bf16)
            cc_ins = [dram.tile([dim, T_CHUNK], f32, name=f"cc_in{i}")
                      for i in range(n_tc)]
            cc_outs = [dram.tile([dim_shard, T_CHUNK], f32, name=f"cc_out{i}")
                       for i in range(n_tc)]

            gwT = gwT_e[:].rearrange("(ko p) h -> p ko h", p=P)
            uwT = uwT_e[:].rearrange("(ko p) h -> p ko h", p=P)
            dwT = dwT_e[:].rearrange("(ho p) m -> p ho m", p=P)
            dwq_r = dwq[:].rearrange("(ho p) m -> p ho m", p=P)
            xT = xT_e[:].rearrange("(ko p) t -> p ko t", p=P)
            hid_r = hid[:].rearrange("(ho p) t -> p ho t", p=P)
            out_r = out_e[:].rearrange("(o p) t -> p o t", p=P)

            def quantize_tile(pool, w_src_ap, wq_dst_ap, thr_col, shape, tag):
                """wq = sign(w) * (|w| > thr), exact f32 compare, bf16 result."""
                wf = pool.tile(shape, f32, tag=f"wf_{tag}", bufs=3)
                nc.sync.dma_start(wf[:], w_src_ap)
                ab = pool.tile(shape, f32, tag=f"ab_{tag}", bufs=2)
                nc.scalar.activation(ab[:], wf[:], mybir.ActivationFunctionType.Abs)
                mask = pool.tile(shape, f32, tag=f"mask_{tag}", bufs=2)
                nc.vector.tensor_scalar(
                    mask[:], ab[:], thr_sb[:, thr_col : thr_col + 1], None,
                    mybir.AluOpType.is_gt,
                )
                sgn = pool.tile(shape, f32, tag=f"sgn_{tag}", bufs=2)
                nc.scalar.sign(sgn[:], wf[:])
                nc.vector.tensor_tensor(
                    wq_dst_ap, mask[:], sgn[:], mybir.AluOpType.mult
                )

            # ---------------- Phase A: gate/up matmuls + SwiGLU ----------------
            with (
                tc.tile_pool(name="pa", bufs=2) as pa,
                tc.tile_pool(name="psA", bufs=4, space="PSUM") as psA,
            ):
                for slab in range(n_slab):
                    hsl = slice(slab * H_SLAB, (slab + 1) * H_SLAB)
                    # quantize this slab of gate/up weights into SBUF (bf16)
                    wq_g = pa.tile([P, n_ko, H_SLAB], bf16, tag="wq_g", bufs=2)
                    wq_u = pa.tile([P, n_ko, H_SLAB], bf16, tag="wq_u", bufs=2)
                    for ko in range(n_ko):
                        quantize_tile(pa, gwT[:, ko, hsl], wq_g[:, ko, :], 0,
                                      [P, H_SLAB], "gu")
                        quantize_tile(pa, uwT[:, ko, hsl], wq_u[:, ko, :], 1,
                                      [P, H_SLAB], "gu")
                    # quantize this slab's rows of down_w to DRAM (bf16)
                    for ho in range(slab * ht_per_slab, (slab + 1) * ht_per_slab):
                        for dsl in range(n_dslab):
                            dsl_sl = slice(dsl * D_SLAB, (dsl + 1) * D_SLAB)
                            wqd = pa.tile([P, D_SLAB], bf16, tag="wqd", bufs=2)
                            quantize_tile(pa, dwT[:, ho, dsl_sl], wqd[:], 2,
                                          [P, D_SLAB], "d")
                            nc.sync.dma_start(dwq_r[:, ho, dsl_sl], wqd[:])

                    for tci in range(n_tc):
                        tsl = slice(tci * T_CHUNK, (tci + 1) * T_CHUNK)
                        xt = pa.tile([P, n_ko, T_CHUNK], bf16, tag="xt", bufs=2)
                        nc.sync.dma_start(xt[:], xT[:, :, tsl])
                        for ht in range(ht_per_slab):
                            hcol = slice(ht * P, (ht + 1) * P)
                            ho_glob = slab * ht_per_slab + ht
                            ps_g = psA.tile([P, T_CHUNK], f32, tag="ps_g")
                            for ko in range(n_ko):
                                nc.tensor.matmul(
                                    ps_g[:], wq_g[:, ko, hcol], xt[:, ko, :],
                                    start=(ko == 0), stop=(ko == n_ko - 1),
                                )
                            ps_u = psA.tile([P, T_CHUNK], f32, tag="ps_u")
                            for ko in range(n_ko):
                                nc.tensor.matmul(
                                    ps_u[:], wq_u[:, ko, hcol], xt[:, ko, :],
                                    start=(ko == 0), stop=(ko == n_ko - 1),
                                )
                            t_silu = pa.tile([P, T_CHUNK], f32, tag="t_silu", bufs=2)
                            nc.scalar.activation(
                                t_silu[:], ps_g[:],
                                mybir.ActivationFunctionType.Silu,
                                scale=gs_sb[:, ho_glob : ho_glob + 1],
                            )
                            t_up = pa.tile([P, T_CHUNK], f32, tag="t_up", bufs=2)
                            nc.scalar.activation(
                                t_up[:], ps_u[:],
                                mybir.ActivationFunctionType.Copy,
                                scale=us_sb[:, ho_glob : ho_glob + 1],
                            )
                            hid_t = pa.tile([P, T_CHUNK], bf16, tag="hid_t", bufs=3)
                            nc.vector.tensor_tensor(
                                hid_t[:], t_silu[:], t_up[:], mybir.AluOpType.mult
                            )
                            nc.sync.dma_start(hid_r[:, ho_glob, tsl], hid_t[:])

            # ---------------- Phase B: down matmul + ReduceScatter ----------------
            with (
                tc.tile_pool(name="pb", bufs=2) as pb,
                tc.tile_pool(name="psB", bufs=4, space="PSUM") as psB,
            ):
                dwq_sb = []
                for dsl in range(n_dslab):
                    dsl_sl = slice(dsl * D_SLAB, (dsl + 1) * D_SLAB)
                    w = pb.tile([P, n_ho, D_SLAB], bf16, tag=f"dwq_sb{dsl}", bufs=1)
                    nc.sync.dma_start(w[:], dwq_r[:, :, dsl_sl])
                    dwq_sb.append(w)

                for tci in range(n_tc):
                    tsl = slice(tci * T_CHUNK, (tci + 1) * T_CHUNK)
                    hid_sb = pb.tile([P, n_ho, T_CHUNK], bf16, tag="hid_sb", bufs=2)
                    nc.sync.dma_start(hid_sb[:], hid_r[:, :, tsl])
                    for dsl in range(n_dslab):
                        for dt in range(dt_per_dslab):
                            dim_tile = dsl * dt_per_dslab + dt
                            dcol = slice(dt * P, (dt + 1) * P)
                            ps = psB.tile([P, T_CHUNK], f32, tag="ps_d")
                            for ho in range(n_ho):
                                nc.tensor.matmul(
                                    ps[:], dwq_sb[dsl][:, ho, dcol], hid_sb[:, ho, :],
                                    start=(ho == 0), stop=(ho == n_ho - 1),
                                )
                            ob = pb.tile([P, T_CHUNK], f32, tag="ob", bufs=4)
                            nc.scalar.copy(ob[:], ps[:])
                            nc.sync.dma_start(
                                cc_ins[tci][dim_tile * P : (dim_tile + 1) * P, :],
                                ob[:],
                            )
                    nc.gpsimd.collective_compute(
                        "ReduceScatter",
                        mybir.AluOpType.add,
                        replica_groups=[list(range(NCORES))],
                        ins=[cc_ins[tci][:].opt()],
                        outs=[cc_outs[tci][:].opt()],
                    )
                    rs_sb = pb.tile([P, n_oo, T_CHUNK], f32, tag="rs_sb", bufs=2)
                    nc.sync.dma_start(
                        rs_sb[:], cc_outs[tci][:].rearrange("(o p) t -> p o t", p=P)
                    )
                    for oo in range(n_oo):
                        nc.vector.tensor_scalar(
                            rs_sb[:, oo, :], rs_sb[:, oo, :],
                            ds_sb[:, oo : oo + 1], None,
                            mybir.AluOpType.mult,
                        )
                    nc.sync.dma_start(out_r[:, :, tsl], rs_sb[:])

    nc.finalize()
    return nc


def _get_nc(d, t_total, h_total, dim):
    key = (d, t_total, h_total, dim)
    if key not in _CACHE:
        _CACHE[key] = _build(*key)
    return _CACHE[key]


def _thresholds(*ws):
    """mean(|w|)*0.7 per matrix, computed with jnp on CPU to match the
    reference's XLA-CPU reduction rounding bit-for-bit."""
    import jax
    import jax.numpy as jnp

    cpu = jax.devices("cpu")[0]
    outs = []
    for w in ws:
        wc = jax.device_put(np.asarray(w), cpu)
        with jax.default_device(cpu):
            thr = jnp.mean(jnp.abs(wc)) * 0.7
        outs.append(np.float32(thr))
    return outs


def prepare(x, gate_w, gate_scale, up_w, up_scale, down_w, down_scale):
    """Host-side prep: thresholds, layout transposes, per-core sharding.
    Returns (nc, in_maps, (B, S, dim))."""
    x = np.asarray(x)
    gate_w = np.asarray(gate_w, dtype=np.float32)
    up_w = np.asarray(up_w, dtype=np.float32)
    down_w = np.asarray(down_w, dtype=np.float32)
    gate_scale = np.asarray(gate_scale, dtype=np.float32)
    up_scale = np.asarray(up_scale, dtype=np.float32)
    down_scale = np.asarray(down_scale, dtype=np.float32)

    B, S, d = x.shape
    t_total = B * S
    h_total = gate_w.shape[0]
    dim = down_w.shape[0]
    h_local = h_total // NCORES
    dim_shard = dim // NCORES

    nc = _get_nc(d, t_total, h_total, dim)

    thr_g, thr_u, thr_d = _thresholds(gate_w, up_w, down_w)
    thr_np = np.tile(np.array([[thr_g, thr_u, thr_d]], np.float32), (P, 1))

    X = x.reshape(t_total, d).astype(np.float32)
    xT = np.ascontiguousarray(X.T).astype(BF16)
    gwT = np.ascontiguousarray(gate_w.T)   # [d, h_total]
    uwT = np.ascontiguousarray(up_w.T)
    dwT = np.ascontiguousarray(down_w.T)   # [h_total, dim]

    in_maps = []
    for c in range(NCORES):
        hsl = slice(c * h_local, (c + 1) * h_local)
        osl = slice(c * dim_shard, (c + 1) * dim_shard)
        in_maps.append({
            "xT": xT,
            "gwT": gwT[:, hsl],
            "uwT": uwT[:, hsl],
            "dwT": dwT[hsl, :],
            "gs": gate_scale[hsl],
            "us": up_scale[hsl],
            "ds": down_scale[osl],
            "thr": thr_np,
        })
    return nc, in_maps, (B, S, dim)


def assemble(results, B, S, dim):
    outT = np.concatenate([results[c]["out"] for c in range(NCORES)], axis=0)
    return np.ascontiguousarray(outT.T).reshape(B, S, dim).astype(np.float32)


def kernel(x, gate_w, gate_scale, up_w, up_scale, down_w, down_scale):
    from concourse.bass_utils import run_bass_kernel_spmd

    nc, in_maps, (B, S, dim) = prepare(
        x, gate_w, gate_scale, up_w, up_scale, down_w, down_scale
    )
    res = run_bass_kernel_spmd(nc, in_maps, list(range(NCORES)), trace=False)
    return assemble(res.results, B, S, dim)


if __name__ == "__main__":
    # small-scale structural self-test against a numpy reference
    rng = np.random.default_rng(0)
    d, t_total, h_total, dim = 512, 1024, 1024, 1024
    B, S = 2, t_total // 2
    x = rng.standard_normal((B, S, d), dtype=np.float32)
    gw = (rng.standard_normal((h_total, d), dtype=np.float32) / np.sqrt(d))
    uw = (rng.standard_normal((h_total, d), dtype=np.float32) / np.sqrt(d))
    dw = (rng.standard_normal((dim, h_total), dtype=np.float32) / np.sqrt(h_total))
    gsc = np.ones((h_total, 1), np.float32)
    usc = np.ones((h_total, 1), np.float32)
    dsc = np.ones((dim, 1), np.float32)

    def np_bitlinear(xf, w, scale):
        thr = np.abs(w).mean() * np.float32(0.7)
        wq = np.sign(w) * (np.abs(w) > thr)
        return xf @ (wq * scale).T

    Xf = x.reshape(-1, d)
    gate = np_bitlinear(Xf, gw, gsc)
    up = np_bitlinear(Xf, uw, usc)
    hidden = gate / (1 + np.exp(-gate)) * up
    exp = np_bitlinear(hidden, dw, dsc).reshape(B, S, dim)

    got = kernel(x=x, gate_w=gw, gate_scale=gsc, up_w=uw, up_scale=usc,
                 down_w=dw, down_scale=dsc)
    err = np.abs(got - exp).max() / np.abs(exp).max()
    print("rel absmax err:", err)
    print("PASS" if err < 5e-3 else "FAIL")


# revision 7
# speedup vs baseline: 2.7829x; 2.7829x over previous
"""BitNet MLP (ternary-quantized SwiGLU) on 8 Trainium2 NeuronCores.

Strategy: tensor-parallel over hidden_dim. Each core owns a 1/8 slice of
gate/up rows and the matching down_proj columns. Activations are kept in
transposed layout [feature, token] on device so every matmul contracts over
the partition dimension with no on-device transposes. Weights are ternarized
on device (mask via fused abs/is_gt on DVE, sign on ACT) into bf16; matmuls
run in bf16 with fp32 PSUM accumulation. The down-proj partial sums are
ReduceScattered across the 8 cores in token chunks, overlapping the
collective with the remaining compute.
"""

import sys

sys.path.insert(0, "/opt/trn_rl_repo")

import numpy as np
import ml_dtypes

BF16 = ml_dtypes.bfloat16
NCORES = 8
P = 128

_CACHE = {}


def _build(d, t_total, h_total, dim, with_collective=True):
    """Build + finalize the SPMD Bass module for the given full dims."""
    import concourse.mybir as mybir
    import concourse.tile as tile
    from concourse import bacc

    f32 = mybir.dt.float32
    bf16 = mybir.dt.bfloat16

    h_local = h_total // NCORES
    dim_shard = dim // NCORES

    T_CHUNK = 512 if t_total % 512 == 0 else 256
    H_SLAB = 256 if h_local % 256 == 0 else 128
    D_SLAB = 512 if dim % 512 == 0 else dim

    n_tc = t_total // T_CHUNK
    n_slab = h_local // H_SLAB
    ht_per_slab = H_SLAB // P
    n_ko = d // P            # contraction tiles for gate/up (over d)
    n_ho = h_local // P      # contraction tiles for down (over h_local)
    n_dslab = dim // D_SLAB
    dt_per_dslab = D_SLAB // P
    n_dim_tiles = dim // P
    n_oo = dim_shard // P    # output row tiles per core

    assert t_total % T_CHUNK == 0 and h_local % H_SLAB == 0
    assert d % P == 0 and dim % D_SLAB == 0 and D_SLAB % P == 0
    assert dim_shard % P == 0

    nc = bacc.Bacc("TRN2", target_bir_lowering=False, debug=False)

    xT_e = nc.dram_tensor("xT", [d, t_total], bf16, kind="ExternalInput")
    gwT_e = nc.dram_tensor("gwT", [d, h_local], f32, kind="ExternalInput")
    uwT_e = nc.dram_tensor("uwT", [d, h_local], f32, kind="ExternalInput")
    dwT_e = nc.dram_tensor("dwT", [h_local, dim], f32, kind="ExternalInput")
    gs_e = nc.dram_tensor("gs", [h_local, 1], f32, kind="ExternalInput")
    us_e = nc.dram_tensor("us", [h_local, 1], f32, kind="ExternalInput")
    ds_e = nc.dram_tensor("ds", [dim_shard, 1], f32, kind="ExternalInput")
    thr_e = nc.dram_tensor("thr", [P, 3], f32, kind="ExternalInput")
    out_e = nc.dram_tensor("out", [dim_shard, t_total], f32, kind="ExternalOutput")

    with tile.TileContext(nc) as tc:
        with (
            tc.tile_pool(name="const", bufs=1) as constp,
            tc.tile_pool(name="dram", bufs=1, space="DRAM") as dram,
        ):
            thr_sb = constp.tile([P, 3], f32)
            nc.sync.dma_start(thr_sb[:], thr_e[:])
            gs_sb = constp.tile([P, n_ho], f32)
            nc.sync.dma_start(gs_sb[:], gs_e[:].rearrange("(o p) u -> p (o u)", p=P))
            us_sb = constp.tile([P, n_ho], f32)
            nc.sync.dma_start(us_sb[:], us_e[:].rearrange("(o p) u -> p (o u)", p=P))
            ds_sb = constp.tile([P, n_oo], f32)
            nc.sync.dma_start(ds_sb[:], ds_e[:].rearrange("(o p) u -> p (o u)", p=P))

            hid = dram.tile([h_local, t_total], bf16)
            dwq4 = dram.tile([n_ho, dim // P, P, P], bf16)
            cc_ins = [dram.tile([dim, T_CHUNK], f32, name=f"cc_in{i}")
                      for i in range(n_tc)]
            cc_outs = [dram.tile([dim_shard, T_CHUNK], f32, name=f"cc_out{i}")
                       for i in range(n_tc)]

            gwT = gwT_e[:].rearrange("(ko p) h -> p ko h", p=P)
            uwT = uwT_e[:].rearrange("(ko p) h -> p ko h", p=P)
            dwT = dwT_e[:].rearrange("(ho p) m -> p ho m", p=P)
            xT = xT_e[:].rearrange("(ko p) t -> p ko t", p=P)
            hid_r = hid[:].rearrange("(ho p) t -> p ho t", p=P)
            out_r = out_e[:].rearrange("(o p) t -> p o t", p=P)

            def quantize_tile(pool, w_src_ap, wq_dst_ap, thr_col, shape, tag):
                """wq = sign(w) * (|w| > thr), exact f32 compare, bf16 result."""
                wf = pool.tile(shape, f32, tag=f"wf_{tag}", bufs=3)
                nc.sync.dma_start(wf[:], w_src_ap)
                ab = pool.tile(shape, f32, tag=f"ab_{tag}", bufs=2)
                nc.scalar.activation(ab[:], wf[:], mybir.ActivationFunctionType.Abs)
                mask = pool.tile(shape, f32, tag=f"mask_{tag}", bufs=2)
                nc.vector.tensor_scalar(
                    mask[:], ab[:], thr_sb[:, thr_col : thr_col + 1], None,
                    mybir.AluOpType.is_gt,
                )
                sgn = pool.tile(shape, f32, tag=f"sgn_{tag}", bufs=2)
                nc.scalar.sign(sgn[:], wf[:])
                nc.vector.tensor_tensor(
                    wq_dst_ap, mask[:], sgn[:], mybir.AluOpType.mult
                )

            # ---------------- Phase A: gate/up matmuls + SwiGLU ----------------
            # lhsT must be a flat [128,128] SBUF tile: 3D-sliced weight APs hit
            # a ~2.3x slower LDWEIGHTS path on HW (302 vs 132 ns/MM measured).
            with (
                tc.tile_pool(name="pa", bufs=2) as pa,
                tc.tile_pool(name="psA", bufs=4, space="PSUM") as psA,
            ):
                for slab in range(n_slab):
                    hsl = slice(slab * H_SLAB, (slab + 1) * H_SLAB)
                    # quantize this slab of gate/up weights into flat SBUF tiles
                    wq_g, wq_u = {}, {}
                    for ko in range(n_ko):
                        for mname, wsrc, wdict, col in (
                            ("g", gwT, wq_g, 0), ("u", uwT, wq_u, 1),
                        ):
                            wf = pa.tile([P, H_SLAB], f32, tag="wf_gu", bufs=3,
                                         name=f"wf_{mname}_{slab}_{ko}")
                            nc.sync.dma_start(wf[:], wsrc[:, ko, hsl])
                            ab = pa.tile([P, H_SLAB], f32, tag="ab_gu", bufs=2,
                                         name=f"ab_{mname}_{slab}_{ko}")
                            nc.scalar.activation(
                                ab[:], wf[:], mybir.ActivationFunctionType.Abs)
                            mask = pa.tile([P, H_SLAB], f32, tag="mask_gu", bufs=2,
                                           name=f"mask_{mname}_{slab}_{ko}")
                            nc.vector.tensor_scalar(
                                mask[:], ab[:], thr_sb[:, col : col + 1], None,
                                mybir.AluOpType.is_gt,
                            )
                            sgn = pa.tile([P, H_SLAB], f32, tag="sgn_gu", bufs=2,
                                          name=f"sgn_{mname}_{slab}_{ko}")
                            nc.scalar.sign(sgn[:], wf[:])
                            for ht in range(ht_per_slab):
                                hc = slice(ht * P, (ht + 1) * P)
                                wt = pa.tile([P, P], bf16,
                                             tag=f"wq{mname}_{ko}_{ht}", bufs=2,
                                             name=f"wq{mname}_{slab}_{ko}_{ht}")
                                nc.vector.tensor_tensor(
                                    wt[:], mask[:, hc], sgn[:, hc],
                                    mybir.AluOpType.mult,
                                )
                                wdict[(ko, ht)] = wt
                    # quantize this slab's rows of down_w to DRAM (bf16),
                    # tile-major so phase B loads are contiguous 32KB blocks
                    for ho in range(slab * ht_per_slab, (slab + 1) * ht_per_slab):
                        for dsl in range(n_dslab):
                            dsl_sl = slice(dsl * D_SLAB, (dsl + 1) * D_SLAB)
                            wqd = pa.tile([P, D_SLAB], bf16, tag="wqd", bufs=2)
                            quantize_tile(pa, dwT[:, ho, dsl_sl], wqd[:], 2,
                                          [P, D_SLAB], "d")
                            for dt in range(dt_per_dslab):
                                nc.sync.dma_start(
                                    dwq4[ho, dsl * dt_per_dslab + dt],
                                    wqd[:, dt * P:(dt + 1) * P],
                                )

                    for tci in range(n_tc):
                        tsl = slice(tci * T_CHUNK, (tci + 1) * T_CHUNK)
                        xt = pa.tile([P, n_ko, T_CHUNK], bf16, tag="xt", bufs=2)
                        nc.sync.dma_start(xt[:], xT[:, :, tsl])
                        for ht in range(ht_per_slab):
                            ho_glob = slab * ht_per_slab + ht
                            ps_g = psA.tile([P, T_CHUNK], f32, tag="ps_g")
                            for ko in range(n_ko):
                                nc.tensor.matmul(
                                    ps_g[:], wq_g[(ko, ht)][:], xt[:, ko, :],
                                    start=(ko == 0), stop=(ko == n_ko - 1),
                                )
                            ps_u = psA.tile([P, T_CHUNK], f32, tag="ps_u")
                            for ko in range(n_ko):
                                nc.tensor.matmul(
                                    ps_u[:], wq_u[(ko, ht)][:], xt[:, ko, :],
                                    start=(ko == 0), stop=(ko == n_ko - 1),
                                )
                            t_silu = pa.tile([P, T_CHUNK], f32, tag="t_silu", bufs=2)
                            nc.scalar.activation(
                                t_silu[:], ps_g[:],
                                mybir.ActivationFunctionType.Silu,
                                scale=gs_sb[:, ho_glob : ho_glob + 1],
                            )
                            t_up = pa.tile([P, T_CHUNK], f32, tag="t_up", bufs=2)
                            nc.scalar.activation(
                                t_up[:], ps_u[:],
                                mybir.ActivationFunctionType.Copy,
                                scale=us_sb[:, ho_glob : ho_glob + 1],
                            )
                            hid_t = pa.tile([P, T_CHUNK], bf16, tag="hid_t", bufs=3)
                            nc.vector.tensor_tensor(
                                hid_t[:], t_silu[:], t_up[:], mybir.AluOpType.mult
                            )
                            nc.sync.dma_start(hid_r[:, ho_glob, tsl], hid_t[:])

            # ---------------- Phase B: down matmul + ReduceScatter ----------------
            with (
                tc.tile_pool(name="pb", bufs=2) as pb,
                tc.tile_pool(name="psB", bufs=4, space="PSUM") as psB,
            ):
                dwq_sb = {}
                for dim_tile in range(n_dim_tiles):
                    for ho in range(n_ho):
                        w2 = pb.tile([P, P], bf16, tag=f"dw_{dim_tile}_{ho}",
                                     bufs=1, name=f"dw_{dim_tile}_{ho}")
                        nc.sync.dma_start(w2[:], dwq4[ho, dim_tile])
                        dwq_sb[(dim_tile, ho)] = w2

                for tci in range(n_tc):
                    tsl = slice(tci * T_CHUNK, (tci + 1) * T_CHUNK)
                    hid_sb = pb.tile([P, n_ho, T_CHUNK], bf16, tag="hid_sb", bufs=2)
                    nc.sync.dma_start(hid_sb[:], hid_r[:, :, tsl])
                    for dim_tile in range(n_dim_tiles):
                            ps = psB.tile([P, T_CHUNK], f32, tag="ps_d")
                            for ho in range(n_ho):
                                nc.tensor.matmul(
                                    ps[:], dwq_sb[(dim_tile, ho)][:], hid_sb[:, ho, :],
                                    start=(ho == 0), stop=(ho == n_ho - 1),
                                )
                            ob = pb.tile([P, T_CHUNK], f32, tag="ob", bufs=4)
                            nc.scalar.copy(ob[:], ps[:])
                            nc.sync.dma_start(
                                cc_ins[tci][dim_tile * P : (dim_tile + 1) * P, :],
                                ob[:],
                            )
                    if with_collective:
                        nc.gpsimd.collective_compute(
                            "ReduceScatter",
                            mybir.AluOpType.add,
                            replica_groups=[list(range(NCORES))],
                            ins=[cc_ins[tci][:].opt()],
                            outs=[cc_outs[tci][:].opt()],
                        )
                    rs_sb = pb.tile([P, n_oo, T_CHUNK], f32, tag="rs_sb", bufs=2)
                    nc.sync.dma_start(
                        rs_sb[:], cc_outs[tci][:].rearrange("(o p) t -> p o t", p=P)
                    )
                    for oo in range(n_oo):
                        nc.vector.tensor_scalar(
                            rs_sb[:, oo, :], rs_sb[:, oo, :],
                            ds_sb[:, oo : oo + 1], None,
                            mybir.AluOpType.mult,
                        )
                    nc.sync.dma_start(out_r[:, :, tsl], rs_sb[:])

    nc.finalize()
    return nc


def _get_nc(d, t_total, h_total, dim, with_collective=True):
    key = (d, t_total, h_total, dim, with_collective)
    if key not in _CACHE:
        _CACHE[key] = _build(d, t_total, h_total, dim, with_collective)
    return _CACHE[key]


def _thresholds(*ws):
    """mean(|w|)*0.7 per matrix, computed with jnp on CPU to match the
    reference's XLA-CPU reduction rounding bit-for-bit."""
    import jax
    import jax.numpy as jnp

    cpu = jax.devices("cpu")[0]
    outs = []
    for w in ws:
        wc = jax.device_put(np.asarray(w), cpu)
        with jax.default_device(cpu):
            thr = jnp.mean(jnp.abs(wc)) * 0.7
        outs.append(np.float32(thr))
    return outs


def prepare(x, gate_w, gate_scale, up_w, up_scale, down_w, down_scale):
    """Host-side prep: thresholds, layout transposes, per-core sharding.
    Returns (nc, in_maps, (B, S, dim))."""
    x = np.asarray(x)
    gate_w = np.asarray(gate_w, dtype=np.float32)
    up_w = np.asarray(up_w, dtype=np.float32)
    down_w = np.asarray(down_w, dtype=np.float32)
    gate_scale = np.asarray(gate_scale, dtype=np.float32)
    up_scale = np.asarray(up_scale, dtype=np.float32)
    down_scale = np.asarray(down_scale, dtype=np.float32)

    B, S, d = x.shape
    t_total = B * S
    h_total = gate_w.shape[0]
    dim = down_w.shape[0]
    h_local = h_total // NCORES
    dim_shard = dim // NCORES

    nc = _get_nc(d, t_total, h_total, dim)

    thr_g, thr_u, thr_d = _thresholds(gate_w, up_w, down_w)
    thr_np = np.tile(np.array([[thr_g, thr_u, thr_d]], np.float32), (P, 1))

    X = x.reshape(t_total, d).astype(np.float32)
    xT = np.ascontiguousarray(X.T).astype(BF16)
    gwT = np.ascontiguousarray(gate_w.T)   # [d, h_total]
    uwT = np.ascontiguousarray(up_w.T)
    dwT = np.ascontiguousarray(down_w.T)   # [h_total, dim]

    in_maps = []
    for c in range(NCORES):
        hsl = slice(c * h_local, (c + 1) * h_local)
        osl = slice(c * dim_shard, (c + 1) * dim_shard)
        in_maps.append({
            "xT": xT,
            "gwT": gwT[:, hsl],
            "uwT": uwT[:, hsl],
            "dwT": dwT[hsl, :],
            "gs": gate_scale[hsl],
            "us": up_scale[hsl],
            "ds": down_scale[osl],
            "thr": thr_np,
        })
    return nc, in_maps, (B, S, dim)


def assemble(results, B, S, dim):
    outT = np.concatenate([results[c]["out"] for c in range(NCORES)], axis=0)
    return np.ascontiguousarray(outT.T).reshape(B, S, dim).astype(np.float32)


def kernel(x, gate_w, gate_scale, up_w, up_scale, down_w, down_scale):
    from concourse.bass_utils import run_bass_kernel_spmd

    nc, in_maps, (B, S, dim) = prepare(
        x, gate_w, gate_scale, up_w, up_scale, down_w, down_scale
    )
    res = run_bass_kernel_spmd(nc, in_maps, list(range(NCORES)), trace=False)
    return assemble(res.results, B, S, dim)


if __name__ == "__main__":
    # small-scale structural self-test against a numpy reference
    rng = np.random.default_rng(0)
    d, t_total, h_total, dim = 512, 1024, 1024, 1024
    B, S = 2, t_total // 2
    x = rng.standard_normal((B, S, d), dtype=np.float32)
    gw = (rng.standard_normal((h_total, d), dtype=np.float32) / np.sqrt(d))
    uw = (rng.standard_normal((h_total, d), dtype=np.float32) / np.sqrt(d))
    dw = (rng.standard_normal((dim, h_total), dtype=np.float32) / np.sqrt(h_total))
    gsc = np.ones((h_total, 1), np.float32)
    usc = np.ones((h_total, 1), np.float32)
    dsc = np.ones((dim, 1), np.float32)

    def np_bitlinear(xf, w, scale):
        thr = np.abs(w).mean() * np.float32(0.7)
        wq = np.sign(w) * (np.abs(w) > thr)
        return xf @ (wq * scale).T

    Xf = x.reshape(-1, d)
    gate = np_bitlinear(Xf, gw, gsc)
    up = np_bitlinear(Xf, uw, usc)
    hidden = gate / (1 + np.exp(-gate)) * up
    exp = np_bitlinear(hidden, dw, dsc).reshape(B, S, dim)

    got = kernel(x=x, gate_w=gw, gate_scale=gsc, up_w=uw, up_scale=usc,
                 down_w=dw, down_scale=dsc)
    err = np.abs(got - exp).max() / np.abs(exp).max()
    print("rel absmax err:", err)
    print("PASS" if err < 5e-3 else "FAIL")
